# revision 37
# baseline (speedup 1.0000x reference)
"""Trainium2 Bass kernel for nn_EngramShortConv (RMSNorm + depthwise dilated
causal conv1d + silu), 8-core SPMD.

  x: [B=4, L=4096, HC=4, D=1024] fp32 -> y same shape/dtype.

Sharding: 16 independent (b, hc) groups, 2 per NeuronCore, zero communication.

v8 ("channel-major everywhere"): the host ships x already transposed to
channel-major [g, d, l] and un-transposes y on unpack, so the device never
transposes anything: no PE transpose pass, no PSUM->SBUF z-copies.

Per core, per 512-token chunk (z := channel-major x tile [128d, t]):
  1. sq = z*z          DVE tensor_tensor fp16 (2x mode), one instr per chunk
  2. ms row            PE: 8 PSUM-accumulated ones-matmuls reduce sq over
                       the channel partitions -> ms[1, t] (1/D in the ones)
  3. r = rsqrt(ms+eps) DVE bit-trick + 1 Newton step on [nbatch, 512] rows
                       batched over several chunks (free-size-bound, so
                       batching makes it ~free)
  4. rbc               PE broadcast-matmul ones[1,128]^T @ r_row -> [128, t]
                       PSUM; ACT Copy casts it to fp16 SBUF
  5. zn = z * rbc      DVE tensor_tensor fp16 (2x), written at halo offset
                       PAD with the previous chunk's tail copied in front
  6. conv              PE: 4 PSUM-accumulated matmuls per 128-channel slab,
                       diag(conv_w[k] * norm_w) @ zn[:, t - 6 + 2k]
  7. silu              ACT Silu reads conv PSUM (two slabs/instr) -> fp16
                       SBUF; DMA out channel-major (host un-transposes)

I/O precision: fp16 in/out on device, fp32 conversion on host. End-to-end
scale-relative error ~3e-3 (budget 2e-2).
"""

import sys

if "/opt/trn_rl_repo" not in sys.path:
    sys.path.insert(0, "/opt/trn_rl_repo")

import numpy as np

B, L, HC, D = 4, 4096, 4, 1024
K, DIL = 4, 2
EPS = 1e-5
PAD = (K - 1) * DIL  # 6
NCORES = 8
NGROUPS = B * HC     # 16
GPC = NGROUPS // NCORES  # 2 groups per core

# tunables
TCH = 512            # tokens per chunk (= conv matmul moving free dim)
RBATCH0 = (1, 2, 3)      # rsqrt row-batch sizes for the first group
RBATCH = (3,)            # and for subsequent groups (max 3: rows at p=32*bi)
SQ_ACT_EVERY = 3     # every n-th chunk computes squares on ACT instead of DVE

_prog_cache = {}


def build_program(L_=L, gpc=GPC, tch=TCH):
    """Build the per-core Bacc program. Same program on all cores (SPMD)."""
    import concourse.bacc as bacc
    import concourse.tile as tile
    from concourse import mybir

    f32 = mybir.dt.float32
    f16 = mybir.dt.float16
    i32 = mybir.dt.int32
    AF = mybir.ActivationFunctionType
    ALU = mybir.AluOpType

    dsub = D // 128
    nchunks = L_ // tch
    assert tch % 128 == 0 and L_ % tch == 0 and D % 128 == 0

    nc = bacc.Bacc()
    # host-swizzled channel-major input:
    # (g, c, p, s, t) = x_cm[g, d = s*128+p, l = c*tch+t]
    xin = nc.declare_dram_parameter("xin", [gpc, nchunks, 128, dsub, tch],
                                    f16, isOutput=False)
    # host-built diag stationaries, partition-major (128 contiguous rows)
    wdg = nc.declare_dram_parameter("wdg", [128, gpc, K, dsub, 128], f16,
                                    isOutput=False)
    yout = nc.declare_dram_parameter("yout", [gpc, nchunks, 128, dsub, tch],
                                     f16, isOutput=True)

    xv = xin[:]
    yv = yout[:]

    # chunk schedule with rsqrt row-batches
    batches = []  # list of [(g, c), ...]
    for g in range(gpc):
        sizes = RBATCH0 if g == 0 else RBATCH
        c = 0
        it = 0
        while c < nchunks:
            n = sizes[it] if it < len(sizes) else sizes[-1]
            n = min(n, nchunks - c)
            batches.append([(g, c + j) for j in range(n)])
            c += n
            it += 1

    with tile.TileContext(nc) as tc:
        with (
            tc.tile_pool(name="pconst", bufs=1) as pconst,
            tc.tile_pool(name="px", bufs=10) as px,
            tc.tile_pool(name="psq", bufs=3) as psq,
            tc.tile_pool(name="pr", bufs=3) as pr,
            tc.tile_pool(name="pz", bufs=4) as pz,
            tc.tile_pool(name="py", bufs=3) as py,
            tc.tile_pool(name="pms", bufs=2, space="PSUM") as pms,
            tc.tile_pool(name="prb", bufs=2, space="PSUM") as prb,
            tc.tile_pool(name="pp2", bufs=2, space="PSUM") as pp2,
        ):
            wsb = pconst.tile([128, gpc, K, dsub, 128], f16)
            nc.scalar.dma_start(out=wsb[:], in_=wdg[:])
            ones = pconst.tile([128, 1], f16)
            nc.vector.memset(ones[:], 1.0 / D)
            onesbc = pconst.tile([128, 128], f16)
            nc.vector.memset(onesbc[:], 1.0)

            # ms rows live at partitions {0, 32, 64} (matmul out rows must).
            # DVE op cost depends only on free size, so the rsqrt chain just
            # runs on the contiguous range [0, 32*(nb-1)+1) -- the rows in
            # between compute garbage that nothing reads.

            def emit_load_stats(g, c, brow, bi):
                """Load chunk + squares + partition-reduce into ms row at
                partition 32*bi (matmul out rows must sit at 0/32/64/96)."""
                xh = px.tile([128, dsub, tch], f16, tag="xh")
                nc.sync.dma_start(out=xh[:], in_=xv[g, c])
                scr = psq.tile([128, dsub, tch], f16, tag="scr")
                if SQ_ACT_EVERY and c % SQ_ACT_EVERY == SQ_ACT_EVERY - 1:
                    nc.scalar.activation(out=scr[:], in_=xh[:],
                                         func=AF.Square)
                else:
                    nc.vector.tensor_tensor(out=scr[:], in0=xh[:],
                                            in1=xh[:], op=ALU.mult)
                p0 = 32 * bi
                for s in range(dsub):
                    nc.tensor.matmul(
                        brow[p0:p0 + 1, :],
                        lhsT=ones[:], rhs=scr[:, s, :],
                        start=(s == 0), stop=(s == dsub - 1))
                return xh

            def emit_rsqrt(brow, nb):
                """r = rsqrt(ms + eps): bit trick + one Newton step on DVE,
                over the contiguous partition range covering rows 32*bi."""
                hi = 32 * (nb - 1) + 1
                v = pr.tile([128, tch], f32, tag="v")
                nc.vector.tensor_scalar(
                    out=v[0:hi], in0=brow[0:hi], scalar1=EPS, scalar2=None,
                    op0=ALU.add)
                r = pr.tile([128, tch], f32, tag="r")
                nc.vector.tensor_scalar(
                    out=r[0:hi].bitcast(i32), in0=v[0:hi].bitcast(i32),
                    scalar1=1, scalar2=None, op0=ALU.arith_shift_right)
                nc.vector.tensor_scalar(
                    out=r[0:hi].bitcast(i32), in0=r[0:hi].bitcast(i32),
                    scalar1=-1, scalar2=0x5F3759DF,
                    op0=ALU.mult, op1=ALU.add)
                yy = pr.tile([128, tch], f32, tag="yy")
                nc.vector.tensor_tensor(
                    out=yy[0:hi], in0=r[0:hi], in1=r[0:hi], op=ALU.mult)
                nc.vector.tensor_tensor(
                    out=yy[0:hi], in0=yy[0:hi], in1=v[0:hi], op=ALU.mult)
                nc.vector.tensor_scalar(
                    out=yy[0:hi], in0=yy[0:hi], scalar1=-0.5, scalar2=1.5,
                    op0=ALU.mult, op1=ALU.add)
                rn = pr.tile([128, tch], f16, tag="rn")
                nc.vector.tensor_tensor(
                    out=rn[0:hi], in0=r[0:hi], in1=yy[0:hi], op=ALU.mult)
                return rn

            zt_prev = None
            # software pipeline: stats for batch i+1 are emitted before the
            # normalize/conv of batch i
            loaded = []   # [(g, c, xh)] for current batch
            brows = []
            prev = None   # (entries, rn) ready for compute

            def emit_batch_stats(batch):
                brow = pms.tile([128, tch], f32, tag="brow")
                entries = []
                for bi, (g, c) in enumerate(batch):
                    xh = emit_load_stats(g, c, brow, bi)
                    entries.append((g, c, xh, bi))
                rn = emit_rsqrt(brow, len(batch))
                return (entries, rn)

            prev = emit_batch_stats(batches[0])
            for bidx in range(len(batches)):
                entries, rn = prev
                if bidx + 1 < len(batches):
                    prev = emit_batch_stats(batches[bidx + 1])

                for (g, c, xh, bi) in entries:
                    # rbc: broadcast r row across 128 partitions via PE
                    p0 = 32 * bi
                    rb = prb.tile([128, tch], f32, tag="rb")
                    nc.tensor.matmul(rb[:], lhsT=onesbc[p0:p0 + 1, :],
                                     rhs=rn[p0:p0 + 1, :],
                                     start=True, stop=True)
                    rbs = pr.tile([128, tch], f16, tag="rbs")
                    nc.scalar.copy(out=rbs[:], in_=rb[:])

                    # normalize into halo'd z tile
                    zt = pz.tile([128, dsub, PAD + tch], f16, tag="zt")
                    if c == 0:
                        nc.vector.memset(zt[:, :, 0:PAD], 0.0)
                    else:
                        nc.vector.tensor_copy(
                            out=zt[:, :, 0:PAD],
                            in_=zt_prev[:, :, tch:tch + PAD])
                    for s in range(dsub):
                        nc.vector.tensor_tensor(
                            out=zt[:, s, PAD:PAD + tch],
                            in0=xh[:, s, :], in1=rbs[:], op=ALU.mult)
                    zt_prev = zt

                    # conv + silu + store
                    yh = py.tile([128, dsub, tch], f16, tag="yh")
                    for si in range(dsub // 2):
                        yp = pp2.tile([128, 2, tch], f32, tag="yp")
                        for sh in range(2):
                            s = 2 * si + sh
                            for k in range(K):
                                nc.tensor.matmul(
                                    yp[:, sh, :],
                                    lhsT=wsb[:, g, k, s, :],
                                    rhs=zt[:, s, k * DIL:k * DIL + tch],
                                    start=(k == 0), stop=(k == K - 1))
                        nc.scalar.activation(
                            out=yh[:, 2 * si:2 * si + 2, :],
                            in_=yp[:], func=AF.Silu)
                        if si == dsub // 4 - 1:
                            nc.gpsimd.dma_start(
                                out=yv[g, c, :, 0:dsub // 2],
                                in_=yh[:, 0:dsub // 2])
                    nc.gpsimd.dma_start(out=yv[g, c, :, dsub // 2:dsub],
                                        in_=yh[:, dsub // 2:dsub])
    nc.compile()
    return nc


def _host_pack(x, norm_weight, conv_weight):
    """Shard inputs across cores; transpose to channel-major and swizzle for
    contiguous DMA; fold norm weight into diag conv stationaries."""
    dsub = D // 128
    nchunks = L // TCH
    xg = np.ascontiguousarray(x.transpose(0, 2, 1, 3)).reshape(NGROUPS, L, D)
    xg = xg.astype(np.float16)
    # channel-major + chunk swizzle: (g, c, p, s, t) = x[g, c*TCH+t, s*128+p]
    xsw = np.ascontiguousarray(
        xg.reshape(NGROUPS, nchunks, TCH, dsub, 128)
        .transpose(0, 1, 4, 3, 2))

    conv_w = conv_weight.reshape(HC, D, K)            # [hc, d, k]
    weff = conv_w * norm_weight[:, :, None]           # [hc, d, k]
    wr = weff.transpose(0, 2, 1).reshape(HC, K, dsub, 128)  # [hc, k, s, p]
    eye = np.eye(128, dtype=np.float32)
    wdiag = (wr[..., None] * eye).astype(np.float16)  # [hc, K, s, p, m]

    in_maps = []
    for i in range(NCORES):
        gs = [i * GPC + j for j in range(GPC)]
        wcore = np.stack([wdiag[g % HC] for g in gs])  # [gpc, K, s, p, m]
        wpm = np.ascontiguousarray(
            wcore.transpose(3, 0, 1, 2, 4))            # [p, gpc, K, s, m]
        in_maps.append({
            "xin": np.ascontiguousarray(xsw[gs[0]:gs[-1] + 1]),
            "wdg": wpm,
        })
    return in_maps


def _host_unpack(results):
    dsub = D // 128
    nchunks = L // TCH
    # yout per core: [gpc, nchunks, 128, dsub, tch] channel-major
    ys = np.concatenate([r["yout"] for r in results], axis=0)
    ys = ys.reshape(B, HC, nchunks, 128, dsub, TCH)
    # [b, hc, c, p, s, t] -> [b, (c t), hc, (s p)]
    y = ys.transpose(0, 2, 5, 1, 4, 3).reshape(B, L, HC, D)
    return np.ascontiguousarray(y.astype(np.float32))


def _get_prog():
    key = (L, GPC, TCH, RBATCH0, RBATCH, SQ_ACT_EVERY)
    if key not in _prog_cache:
        _prog_cache[key] = build_program()
    return _prog_cache[key]


def kernel(x, norm_weight, conv_weight, _trace=False, _trace_kwargs=None):
    from concourse.bass_utils import run_bass_kernel_spmd

    x = np.asarray(x, dtype=np.float32)
    norm_weight = np.asarray(norm_weight, dtype=np.float32)
    conv_weight = np.asarray(conv_weight, dtype=np.float32)

    nc = _get_prog()
    in_maps = _host_pack(x, norm_weight, conv_weight)
    res = run_bass_kernel_spmd(
        nc, in_maps, list(range(NCORES)),
        trace=_trace, **(_trace_kwargs or {}))
    out = _host_unpack(res.results)
    if _trace:
        return out, res
    return out


# revision 39
# speedup vs baseline: 1.1745x; 1.1745x over previous
"""Trainium2 Bass kernel for nn_EngramShortConv (RMSNorm + depthwise dilated
causal conv1d + silu), 8-core SPMD.

  x: [B=4, L=4096, HC=4, D=1024] fp32 -> y same shape/dtype.

Sharding: 16 independent (b, hc) groups, 2 per NeuronCore, zero communication.

v3:
  - Output written CHANNEL-MAJOR straight from conv/silu PSUM; host does the
    final un-transpose during unpack (deletes the old PE transpose-back pass
    and its PSUM->SBUF copies).
  - All HBM layouts host-swizzled so every DMA is 128 partitions x 8KB
    contiguous (128 descriptors instead of 512-1024 small ones).
  - Conv diag stationaries built on device from compact per-channel weight
    columns (kills a 2MB / 8192-descriptor weight load).
  - Stats (squares -> rsqrt -> diag(r)) run per *chunk* and are emitted one
    pair ahead so the PE never waits on them.

Per core, per 512-token chunk:
  1. stats: x^2 with 1/D folded accumulates to ms per token (engine per
     128-token block set by SQ_ENGINES); r = rsqrt(ms+eps) via bit-trick +
     1 Newton step on DVE (no ACT table swaps).
  2. pass1 (PE): Z[d, t] = X_blk^T @ diag(r) per 128x128 block -- transpose
     to channel-major with the RMSNorm scale folded in. DVE/ACT copy
     PSUM -> SBUF fp16 (two slabs per instruction) with a 6-column halo
     from the previous chunk.
  3. pass2 (PE): depthwise conv as 4 PSUM-accumulated matmuls
     diag(conv_w[k] * norm_w) @ Z[:, t - 6 + 2k].
  4. ACT Silu reads conv PSUM -> fp16 SBUF; DMA out channel-major.

I/O precision: host casts x to fp16 (halves input DMA); device returns fp16
y upcast to fp32 on host. End-to-end scale-relative error ~3e-3.
"""

import sys

if "/opt/trn_rl_repo" not in sys.path:
    sys.path.insert(0, "/opt/trn_rl_repo")

import numpy as np

B, L, HC, D = 4, 4096, 4, 1024
K, DIL = 4, 2
EPS = 1e-5
PAD = (K - 1) * DIL  # 6
NCORES = 8
NGROUPS = B * HC     # 16
GPC = NGROUPS // NCORES  # 2 groups per core

# tunables
TCH = 512            # tokens per chunk (= matmul moving free dim)
CPAIR = 2            # chunks per conv pairing (shares conv ldweights)
SQ_ENGINES = ("vector", "act", "vector", "vector")  # even chunks; odd flipped
ZCOPY_ACT = 1        # of 4 two-slab zcopy units per chunk, how many on ACT

_prog_cache = {}


def build_program(L_=L, gpc=GPC, tch=TCH, cpair=CPAIR,
                  sq_engines=SQ_ENGINES, zcopy_act=ZCOPY_ACT):
    """Build the per-core Bacc program. Same program on all cores (SPMD)."""
    import concourse.bacc as bacc
    import concourse.tile as tile
    from concourse import mybir

    f32 = mybir.dt.float32
    f16 = mybir.dt.float16
    i32 = mybir.dt.int32
    AF = mybir.ActivationFunctionType
    ALU = mybir.AluOpType

    nblk = tch // 128
    dsub = D // 128
    nchunks = L_ // tch
    assert tch % 128 == 0 and L_ % tch == 0 and D % 128 == 0

    nc = bacc.Bacc()
    # host-swizzled input: (g, c, p, blk, d) = x[g, c*tch + blk*128 + p, d]
    xin = nc.declare_dram_parameter("xin", [gpc, nchunks, 128, nblk, D], f16,
                                    isOutput=False)
    # host-built diag stationaries, partition-major so the DMA is 128
    # contiguous 16KB rows: (p, g, k, s, m) = diag(w_eff)[g, k, s][p, m]
    wdg = nc.declare_dram_parameter("wdg", [128, gpc, K, dsub, 128], f16,
                                    isOutput=False)
    idn = nc.declare_dram_parameter("idn", [128, 128], f16, isOutput=False)
    # channel-major output: (g, c, p, s, t) = y[g, d=s*128+p, l=c*tch+t]
    yout = nc.declare_dram_parameter("yout", [gpc, nchunks, 128, dsub, tch],
                                     f16, isOutput=True)

    xv = xin[:]
    yv = yout[:]

    with tile.TileContext(nc) as tc:
        with (
            tc.tile_pool(name="pconst", bufs=1) as pconst,
            tc.tile_pool(name="px", bufs=10) as px,
            tc.tile_pool(name="pstat", bufs=4) as pstat,
            tc.tile_pool(name="pz", bufs=5) as pz,
            tc.tile_pool(name="py", bufs=4) as py,
            tc.tile_pool(name="pp1", bufs=2, space="PSUM") as pp1,
            tc.tile_pool(name="pp2", bufs=2, space="PSUM") as pp2,
        ):
            ident = pconst.tile([128, 128], f16)
            nc.scalar.dma_start(out=ident[:], in_=idn[:])
            wsb = pconst.tile([128, gpc, K, dsub, 128], f16)
            nc.scalar.dma_start(out=wsb[:], in_=wdg[:])

            def emit_stats(g, c):
                """Load one chunk + stats + r + drt, emitted a pair ahead of
                the heavy compute so diag(r) never gates PE."""
                xh = px.tile([128, nblk, D], f16, tag="xh")
                nc.sync.dma_start(out=xh[:], in_=xv[g, c])

                ssq = pstat.tile([128, nblk], f32, tag="ssq")
                for blk in range(nblk):
                    eng = sq_engines[blk % len(sq_engines)]
                    if c % 2 == 1:
                        eng = "act" if eng == "vector" else "vector"
                    scr = pstat.tile([128, D], f16, tag="scr")
                    if eng == "act":
                        nc.scalar.activation(
                            out=scr[:], in_=xh[:, blk, :],
                            func=AF.Square, scale=float(D) ** -0.5,
                            accum_out=ssq[:, blk:blk + 1])
                    else:
                        nc.vector.scalar_tensor_tensor(
                            out=scr[:], in0=xh[:, blk, :],
                            scalar=1.0 / D, in1=xh[:, blk, :],
                            op0=ALU.mult, op1=ALU.mult,
                            accum_out=ssq[:, blk:blk + 1])
                # r = rsqrt(ms+eps): bit trick + 1 Newton step on
                # DVE (keeps Sqrt out of ACT -> zero table swaps)
                v = pstat.tile([128, nblk], f32, tag="v")
                nc.vector.tensor_scalar(
                    out=v[:], in0=ssq[:], scalar1=EPS, scalar2=None,
                    op0=ALU.add)
                r = pstat.tile([128, nblk], f32, tag="r")
                nc.vector.tensor_scalar(
                    out=r[:].bitcast(i32), in0=v[:].bitcast(i32),
                    scalar1=1, scalar2=None, op0=ALU.arith_shift_right)
                nc.vector.tensor_scalar(
                    out=r[:].bitcast(i32), in0=r[:].bitcast(i32),
                    scalar1=-1, scalar2=0x5F3759DF,
                    op0=ALU.mult, op1=ALU.add)
                yy = pstat.tile([128, nblk], f32, tag="yy")
                nc.vector.tensor_tensor(
                    out=yy[:], in0=r[:], in1=r[:], op=ALU.mult)
                nc.vector.tensor_tensor(
                    out=yy[:], in0=yy[:], in1=v[:], op=ALU.mult)
                nc.vector.tensor_scalar(
                    out=yy[:], in0=yy[:], scalar1=-0.5, scalar2=1.5,
                    op0=ALU.mult, op1=ALU.add)
                rn = pstat.tile([128, nblk], f32, tag="rn")
                nc.vector.tensor_tensor(
                    out=rn[:], in0=r[:], in1=yy[:], op=ALU.mult)

                drt = pstat.tile([128, nblk, 128], f16, tag="drt")
                for blk in range(nblk):
                    nc.vector.tensor_scalar_mul(
                        out=drt[:, blk, :], in0=ident[:],
                        scalar1=rn[:, blk:blk + 1])
                return (xh, drt)

            zt_prev = None
            pair_keys = [(g, c0) for g in range(gpc)
                         for c0 in range(0, nchunks, cpair)]
            pending = [emit_stats(pair_keys[0][0], pair_keys[0][1] + j)
                       for j in range(cpair)]
            for pidx, (g, c0) in enumerate(pair_keys):
                cs = list(range(c0, min(c0 + cpair, nchunks)))
                ncs = len(cs)
                cur = pending
                if pidx + 1 < len(pair_keys):
                    g2, c2 = pair_keys[pidx + 1]
                    pending = [emit_stats(g2, c2 + j) for j in range(cpair)]

                # ---- pass1 per chunk: Z[d, t] = X^T diag(r) ----
                zts = []
                for j, c in enumerate(cs):
                    xh, drt = cur[j]
                    zt = pz.tile([128, dsub, PAD + tch], f16, tag="zt")
                    if c == 0:
                        nc.vector.memset(zt[:, :, 0:PAD], 0.0)
                    else:
                        nc.vector.tensor_copy(
                            out=zt[:, :, 0:PAD],
                            in_=zt_prev[:, :, tch:tch + PAD])
                    for si in range(dsub // 2):
                        zp = pp1.tile([128, 2, tch], f32, tag="zp")
                        for sh in range(2):
                            s = 2 * si + sh
                            for blk in range(nblk):
                                nc.tensor.matmul(
                                    zp[:, sh, blk * 128:(blk + 1) * 128],
                                    lhsT=xh[:, blk, s * 128:(s + 1) * 128],
                                    rhs=drt[:, blk, :],
                                    start=True, stop=True)
                        dst = zt[:, 2 * si:2 * si + 2, PAD:PAD + tch]
                        if si < zcopy_act:
                            nc.scalar.copy(out=dst, in_=zp[:])
                        else:
                            nc.vector.tensor_copy(out=dst, in_=zp[:])
                    zt_prev = zt
                    zts.append(zt)

                # ---- pass2 paired: conv matmuls share ldweights;
                #      silu writes fp16 channel-major, DMA straight out
                for j, c in enumerate(cs):
                    yh = py.tile([128, dsub, tch], f16, tag="yh")
                    for si in range(dsub // 2):
                        yp = pp2.tile([128, 2, tch], f32, tag="yp")
                        for sh in range(2):
                            s = 2 * si + sh
                            for k in range(K):
                                nc.tensor.matmul(
                                    yp[:, sh, :],
                                    lhsT=wsb[:, g, k, s, :],
                                    rhs=zts[j][:, s, k * DIL:k * DIL + tch],
                                    start=(k == 0), stop=(k == K - 1))
                        nc.scalar.activation(
                            out=yh[:, 2 * si:2 * si + 2, :],
                            in_=yp[:], func=AF.Silu)
                        if si == dsub // 4 - 1:
                            nc.gpsimd.dma_start(
                                out=yv[g, c, :, 0:dsub // 2],
                                in_=yh[:, 0:dsub // 2])
                    nc.gpsimd.dma_start(out=yv[g, c, :, dsub // 2:dsub],
                                        in_=yh[:, dsub // 2:dsub])
    nc.compile()
    return nc


def _host_pack(x, norm_weight, conv_weight):
    """Shard inputs across cores; swizzle for contiguous DMA; fold norm
    weight into compact per-channel conv weight columns."""
    dsub = D // 128
    nblk = TCH // 128
    nchunks = L // TCH
    xg = np.ascontiguousarray(x.transpose(0, 2, 1, 3)).reshape(NGROUPS, L, D)
    xg = xg.astype(np.float16)
    # (g, c, p, blk, d) = x[g, c*tch + blk*128 + p, d]
    xsw = np.ascontiguousarray(
        xg.reshape(NGROUPS, nchunks, nblk, 128, D).transpose(0, 1, 3, 2, 4))

    conv_w = conv_weight.reshape(HC, D, K)            # [hc, d, k]
    weff = conv_w * norm_weight[:, :, None]           # [hc, d, k]
    wr = weff.transpose(0, 2, 1).reshape(HC, K, dsub, 128)  # [hc, k, s, p]
    eye = np.eye(128, dtype=np.float32)
    wdiag = (wr[..., None] * eye).astype(np.float16)  # [hc, K, s, p, m]
    idn = np.eye(128, dtype=np.float16)

    in_maps = []
    for i in range(NCORES):
        gs = [i * GPC + j for j in range(GPC)]
        wcore = np.stack([wdiag[g % HC] for g in gs])  # [gpc, K, s, p, m]
        wpm = np.ascontiguousarray(
            wcore.transpose(3, 0, 1, 2, 4))            # [p, gpc, K, s, m]
        in_maps.append({
            "xin": np.ascontiguousarray(xsw[gs[0]:gs[-1] + 1]),
            "wdg": wpm,
            "idn": idn,
        })
    return in_maps


def _host_unpack(results):
    dsub = D // 128
    nchunks = L // TCH
    # yout per core: [gpc, nchunks, 128, dsub, tch] channel-major
    ys = np.concatenate([r["yout"] for r in results], axis=0)
    ys = ys.reshape(B, HC, nchunks, 128, dsub, TCH)
    # [b, hc, c, p, s, t] -> [b, (c t), hc, (s p)]
    y = ys.transpose(0, 2, 5, 1, 4, 3).reshape(B, L, HC, D)
    return np.ascontiguousarray(y.astype(np.float32))


def _get_prog():
    key = (L, GPC, TCH, CPAIR, SQ_ENGINES, ZCOPY_ACT)
    if key not in _prog_cache:
        _prog_cache[key] = build_program()
    return _prog_cache[key]


def kernel(x, norm_weight, conv_weight, _trace=False, _trace_kwargs=None):
    from concourse.bass_utils import run_bass_kernel_spmd

    x = np.asarray(x, dtype=np.float32)
    norm_weight = np.asarray(norm_weight, dtype=np.float32)
    conv_weight = np.asarray(conv_weight, dtype=np.float32)

    nc = _get_prog()
    in_maps = _host_pack(x, norm_weight, conv_weight)
    res = run_bass_kernel_spmd(
        nc, in_maps, list(range(NCORES)),
        trace=_trace, **(_trace_kwargs or {}))
    out = _host_unpack(res.results)
    if _trace:
        return out, res
    return out


# revision 40
# speedup vs baseline: 1.1831x; 1.0073x over previous
"""Trainium2 Bass kernel for nn_EngramShortConv (RMSNorm + depthwise dilated
causal conv1d + silu), 8-core SPMD.

  x: [B=4, L=4096, HC=4, D=1024] fp32 -> y same shape/dtype.

Sharding: 16 independent (b, hc) groups, 2 per NeuronCore, zero communication.

v3:
  - Output written CHANNEL-MAJOR straight from conv/silu PSUM; host does the
    final un-transpose during unpack (deletes the old PE transpose-back pass
    and its PSUM->SBUF copies).
  - All HBM layouts host-swizzled so every DMA is 128 partitions x 8KB
    contiguous (128 descriptors instead of 512-1024 small ones).
  - Conv diag stationaries built on device from compact per-channel weight
    columns (kills a 2MB / 8192-descriptor weight load).
  - Stats (squares -> rsqrt -> diag(r)) run per *chunk* and are emitted one
    pair ahead so the PE never waits on them.

Per core, per 512-token chunk:
  1. stats: x^2 with 1/D folded accumulates to ms per token (engine per
     128-token block set by SQ_ENGINES); r = rsqrt(ms+eps) via bit-trick +
     1 Newton step on DVE (no ACT table swaps).
  2. pass1 (PE): Z[d, t] = X_blk^T @ diag(r) per 128x128 block -- transpose
     to channel-major with the RMSNorm scale folded in. DVE/ACT copy
     PSUM -> SBUF fp16 (two slabs per instruction) with a 6-column halo
     from the previous chunk.
  3. pass2 (PE): depthwise conv as 4 PSUM-accumulated matmuls
     diag(conv_w[k] * norm_w) @ Z[:, t - 6 + 2k].
  4. ACT Silu reads conv PSUM -> fp16 SBUF; DMA out channel-major.

I/O precision: host casts x to fp16 (halves input DMA); device returns fp16
y upcast to fp32 on host. End-to-end scale-relative error ~3e-3.
"""

import sys

if "/opt/trn_rl_repo" not in sys.path:
    sys.path.insert(0, "/opt/trn_rl_repo")

import numpy as np

B, L, HC, D = 4, 4096, 4, 1024
K, DIL = 4, 2
EPS = 1e-5
PAD = (K - 1) * DIL  # 6
NCORES = 8
NGROUPS = B * HC     # 16
GPC = NGROUPS // NCORES  # 2 groups per core

# tunables
TCH = 512            # tokens per chunk (= matmul moving free dim)
CPAIR = 2            # chunks per conv pairing (shares conv ldweights)
SQ_ENGINES = ("vector", "act", "vector", "vector")  # even chunks; odd flipped
ZCOPY_ACT = 1        # of 4 two-slab zcopy units per chunk, how many on ACT

_prog_cache = {}


def build_program(L_=L, gpc=GPC, tch=TCH, cpair=CPAIR,
                  sq_engines=SQ_ENGINES, zcopy_act=ZCOPY_ACT):
    """Build the per-core Bacc program. Same program on all cores (SPMD)."""
    import concourse.bacc as bacc
    import concourse.tile as tile
    from concourse import mybir

    f32 = mybir.dt.float32
    f16 = mybir.dt.float16
    i32 = mybir.dt.int32
    AF = mybir.ActivationFunctionType
    ALU = mybir.AluOpType

    nblk = tch // 128
    dsub = D // 128
    nchunks = L_ // tch
    assert tch % 128 == 0 and L_ % tch == 0 and D % 128 == 0

    nc = bacc.Bacc()
    # host-swizzled input: (g, c, p, blk, d) = x[g, c*tch + blk*128 + p, d]
    xin = nc.declare_dram_parameter("xin", [gpc, nchunks, 128, nblk, D], f16,
                                    isOutput=False)
    # host-built diag stationaries, partition-major so the DMA is 128
    # contiguous 16KB rows: (p, g, k, s, m) = diag(w_eff)[g, k, s][p, m]
    wdg = nc.declare_dram_parameter("wdg", [128, gpc, K, dsub, 128], f16,
                                    isOutput=False)
    idn = nc.declare_dram_parameter("idn", [128, 128], f16, isOutput=False)
    # channel-major output: (g, c, p, s, t) = y[g, d=s*128+p, l=c*tch+t]
    yout = nc.declare_dram_parameter("yout", [gpc, nchunks, 128, dsub, tch],
                                     f16, isOutput=True)

    xv = xin[:]
    yv = yout[:]

    with tile.TileContext(nc) as tc:
        with (
            tc.tile_pool(name="pconst", bufs=1) as pconst,
            tc.tile_pool(name="px", bufs=10) as px,
            tc.tile_pool(name="pstat", bufs=4) as pstat,
            tc.tile_pool(name="pz", bufs=5) as pz,
            tc.tile_pool(name="py", bufs=4) as py,
            tc.tile_pool(name="pp1", bufs=2, space="PSUM") as pp1,
            tc.tile_pool(name="pp2", bufs=2, space="PSUM") as pp2,
        ):
            ident = pconst.tile([128, 128], f16)
            nc.scalar.dma_start(out=ident[:], in_=idn[:])
            wsb = pconst.tile([128, gpc, K, dsub, 128], f16)
            nc.scalar.dma_start(out=wsb[:], in_=wdg[:])

            def emit_stats(g, c):
                """Load one chunk + stats + r + drt, emitted a pair ahead of
                the heavy compute so diag(r) never gates PE."""
                xh = px.tile([128, nblk, D], f16, tag="xh")
                nc.sync.dma_start(out=xh[:], in_=xv[g, c])

                ssq = pstat.tile([128, nblk], f32, tag="ssq")
                odd_engines = ("act", "vector", "vector", "act")
                for blk in range(nblk):
                    eng = (sq_engines if c % 2 == 0
                           else odd_engines)[blk % len(sq_engines)]
                    scr = pstat.tile([128, D], f16, tag="scr")
                    if eng == "act":
                        nc.scalar.activation(
                            out=scr[:], in_=xh[:, blk, :],
                            func=AF.Square, scale=float(D) ** -0.5,
                            accum_out=ssq[:, blk:blk + 1])
                    else:
                        nc.vector.scalar_tensor_tensor(
                            out=scr[:], in0=xh[:, blk, :],
                            scalar=1.0 / D, in1=xh[:, blk, :],
                            op0=ALU.mult, op1=ALU.mult,
                            accum_out=ssq[:, blk:blk + 1])
                # r = rsqrt(ms+eps): bit trick + 1 Newton step on
                # DVE (keeps Sqrt out of ACT -> zero table swaps)
                v = pstat.tile([128, nblk], f32, tag="v")
                nc.vector.tensor_scalar(
                    out=v[:], in0=ssq[:], scalar1=EPS, scalar2=None,
                    op0=ALU.add)
                r = pstat.tile([128, nblk], f32, tag="r")
                nc.vector.tensor_scalar(
                    out=r[:].bitcast(i32), in0=v[:].bitcast(i32),
                    scalar1=1, scalar2=None, op0=ALU.arith_shift_right)
                nc.vector.tensor_scalar(
                    out=r[:].bitcast(i32), in0=r[:].bitcast(i32),
                    scalar1=-1, scalar2=0x5F3759DF,
                    op0=ALU.mult, op1=ALU.add)
                yy = pstat.tile([128, nblk], f32, tag="yy")
                nc.vector.tensor_tensor(
                    out=yy[:], in0=r[:], in1=r[:], op=ALU.mult)
                nc.vector.tensor_tensor(
                    out=yy[:], in0=yy[:], in1=v[:], op=ALU.mult)
                nc.vector.tensor_scalar(
                    out=yy[:], in0=yy[:], scalar1=-0.5, scalar2=1.5,
                    op0=ALU.mult, op1=ALU.add)
                rn = pstat.tile([128, nblk], f32, tag="rn")
                nc.vector.tensor_tensor(
                    out=rn[:], in0=r[:], in1=yy[:], op=ALU.mult)

                drt = pstat.tile([128, nblk, 128], f16, tag="drt")
                for blk in range(nblk):
                    nc.vector.tensor_scalar_mul(
                        out=drt[:, blk, :], in0=ident[:],
                        scalar1=rn[:, blk:blk + 1])
                return (xh, drt)

            zt_prev = None
            pair_keys = [(g, c0) for g in range(gpc)
                         for c0 in range(0, nchunks, cpair)]
            pending = [emit_stats(pair_keys[0][0], pair_keys[0][1] + j)
                       for j in range(cpair)]
            for pidx, (g, c0) in enumerate(pair_keys):
                cs = list(range(c0, min(c0 + cpair, nchunks)))
                ncs = len(cs)
                cur = pending
                if pidx + 1 < len(pair_keys):
                    g2, c2 = pair_keys[pidx + 1]
                    pending = [emit_stats(g2, c2 + j) for j in range(cpair)]

                # ---- pass1 per chunk: Z[d, t] = X^T diag(r) ----
                zts = []
                for j, c in enumerate(cs):
                    xh, drt = cur[j]
                    zt = pz.tile([128, dsub, PAD + tch], f16, tag="zt")
                    if c == 0:
                        nc.vector.memset(zt[:, :, 0:PAD], 0.0)
                    else:
                        nc.vector.tensor_copy(
                            out=zt[:, :, 0:PAD],
                            in_=zt_prev[:, :, tch:tch + PAD])
                    for si in range(dsub // 2):
                        zp = pp1.tile([128, 2, tch], f32, tag="zp")
                        for sh in range(2):
                            s = 2 * si + sh
                            for blk in range(nblk):
                                nc.tensor.matmul(
                                    zp[:, sh, blk * 128:(blk + 1) * 128],
                                    lhsT=xh[:, blk, s * 128:(s + 1) * 128],
                                    rhs=drt[:, blk, :],
                                    start=True, stop=True)
                        dst = zt[:, 2 * si:2 * si + 2, PAD:PAD + tch]
                        if si < zcopy_act:
                            nc.scalar.copy(out=dst, in_=zp[:])
                        else:
                            nc.vector.tensor_copy(out=dst, in_=zp[:])
                    zt_prev = zt
                    zts.append(zt)

                # ---- pass2 paired: conv matmuls share ldweights;
                #      silu writes fp16 channel-major, DMA straight out
                for j, c in enumerate(cs):
                    yh = py.tile([128, dsub, tch], f16, tag="yh")
                    for si in range(dsub // 2):
                        yp = pp2.tile([128, 2, tch], f32, tag="yp")
                        for sh in range(2):
                            s = 2 * si + sh
                            for k in range(K):
                                nc.tensor.matmul(
                                    yp[:, sh, :],
                                    lhsT=wsb[:, g, k, s, :],
                                    rhs=zts[j][:, s, k * DIL:k * DIL + tch],
                                    start=(k == 0), stop=(k == K - 1))
                        nc.scalar.activation(
                            out=yh[:, 2 * si:2 * si + 2, :],
                            in_=yp[:], func=AF.Silu)
                        if si == dsub // 4 - 1:
                            nc.gpsimd.dma_start(
                                out=yv[g, c, :, 0:dsub // 2],
                                in_=yh[:, 0:dsub // 2])
                    nc.gpsimd.dma_start(out=yv[g, c, :, dsub // 2:dsub],
                                        in_=yh[:, dsub // 2:dsub])
    nc.compile()
    return nc


def _host_pack(x, norm_weight, conv_weight):
    """Shard inputs across cores; swizzle for contiguous DMA; fold norm
    weight into compact per-channel conv weight columns."""
    dsub = D // 128
    nblk = TCH // 128
    nchunks = L // TCH
    xg = np.ascontiguousarray(x.transpose(0, 2, 1, 3)).reshape(NGROUPS, L, D)
    xg = xg.astype(np.float16)
    # (g, c, p, blk, d) = x[g, c*tch + blk*128 + p, d]
    xsw = np.ascontiguousarray(
        xg.reshape(NGROUPS, nchunks, nblk, 128, D).transpose(0, 1, 3, 2, 4))

    conv_w = conv_weight.reshape(HC, D, K)            # [hc, d, k]
    weff = conv_w * norm_weight[:, :, None]           # [hc, d, k]
    wr = weff.transpose(0, 2, 1).reshape(HC, K, dsub, 128)  # [hc, k, s, p]
    eye = np.eye(128, dtype=np.float32)
    wdiag = (wr[..., None] * eye).astype(np.float16)  # [hc, K, s, p, m]
    idn = np.eye(128, dtype=np.float16)

    in_maps = []
    for i in range(NCORES):
        gs = [i * GPC + j for j in range(GPC)]
        wcore = np.stack([wdiag[g % HC] for g in gs])  # [gpc, K, s, p, m]
        wpm = np.ascontiguousarray(
            wcore.transpose(3, 0, 1, 2, 4))            # [p, gpc, K, s, m]
        in_maps.append({
            "xin": np.ascontiguousarray(xsw[gs[0]:gs[-1] + 1]),
            "wdg": wpm,
            "idn": idn,
        })
    return in_maps


def _host_unpack(results):
    dsub = D // 128
    nchunks = L // TCH
    # yout per core: [gpc, nchunks, 128, dsub, tch] channel-major
    ys = np.concatenate([r["yout"] for r in results], axis=0)
    ys = ys.reshape(B, HC, nchunks, 128, dsub, TCH)
    # [b, hc, c, p, s, t] -> [b, (c t), hc, (s p)]
    y = ys.transpose(0, 2, 5, 1, 4, 3).reshape(B, L, HC, D)
    return np.ascontiguousarray(y.astype(np.float32))


def _get_prog():
    key = (L, GPC, TCH, CPAIR, SQ_ENGINES, ZCOPY_ACT)
    if key not in _prog_cache:
        _prog_cache[key] = build_program()
    return _prog_cache[key]


def kernel(x, norm_weight, conv_weight, _trace=False, _trace_kwargs=None):
    from concourse.bass_utils import run_bass_kernel_spmd

    x = np.asarray(x, dtype=np.float32)
    norm_weight = np.asarray(norm_weight, dtype=np.float32)
    conv_weight = np.asarray(conv_weight, dtype=np.float32)

    nc = _get_prog()
    in_maps = _host_pack(x, norm_weight, conv_weight)
    res = run_bass_kernel_spmd(
        nc, in_maps, list(range(NCORES)),
        trace=_trace, **(_trace_kwargs or {}))
    out = _host_unpack(res.results)
    if _trace:
        return out, res
    return out


# revision 41
# speedup vs baseline: 1.1837x; 1.0005x over previous
"""Trainium2 Bass kernel for nn_EngramShortConv (RMSNorm + depthwise dilated
causal conv1d + silu), 8-core SPMD.

  x: [B=4, L=4096, HC=4, D=1024] fp32 -> y same shape/dtype.

Sharding: 16 independent (b, hc) groups, 2 per NeuronCore, zero communication.

v3:
  - Output written CHANNEL-MAJOR straight from conv/silu PSUM; host does the
    final un-transpose during unpack (deletes the old PE transpose-back pass
    and its PSUM->SBUF copies).
  - All HBM layouts host-swizzled so every DMA is 128 partitions x 8KB
    contiguous (128 descriptors instead of 512-1024 small ones).
  - Conv diag stationaries built on device from compact per-channel weight
    columns (kills a 2MB / 8192-descriptor weight load).
  - Stats (squares -> rsqrt -> diag(r)) run per *chunk* and are emitted one
    pair ahead so the PE never waits on them.

Per core, per 512-token chunk:
  1. stats: x^2 with 1/D folded accumulates to ms per token (engine per
     128-token block set by SQ_ENGINES); r = rsqrt(ms+eps) via bit-trick +
     1 Newton step on DVE (no ACT table swaps).
  2. pass1 (PE): Z[d, t] = X_blk^T @ diag(r) per 128x128 block -- transpose
     to channel-major with the RMSNorm scale folded in. DVE/ACT copy
     PSUM -> SBUF fp16 (two slabs per instruction) with a 6-column halo
     from the previous chunk.
  3. pass2 (PE): depthwise conv as 4 PSUM-accumulated matmuls
     diag(conv_w[k] * norm_w) @ Z[:, t - 6 + 2k].
  4. ACT Silu reads conv PSUM -> fp16 SBUF; DMA out channel-major.

I/O precision: host casts x to fp16 (halves input DMA); device returns fp16
y upcast to fp32 on host. End-to-end scale-relative error ~3e-3.
"""

import sys

if "/opt/trn_rl_repo" not in sys.path:
    sys.path.insert(0, "/opt/trn_rl_repo")

import numpy as np

B, L, HC, D = 4, 4096, 4, 1024
K, DIL = 4, 2
EPS = 1e-5
PAD = (K - 1) * DIL  # 6
NCORES = 8
NGROUPS = B * HC     # 16
GPC = NGROUPS // NCORES  # 2 groups per core

# tunables
TCH = 512            # tokens per chunk (= matmul moving free dim)
CPAIR = 2            # chunks per conv pairing (shares conv ldweights)
SQ_ENGINES = ("vector", "act", "vector", "vector")  # even chunks (odd use a
# different DVE/ACT interleave so consecutive chunks' stats can't collide on
# one engine queue; see odd_engines below)
ZCOPY_ACT = 1        # of 4 two-slab zcopy units per chunk, how many on ACT

_prog_cache = {}


def build_program(L_=L, gpc=GPC, tch=TCH, cpair=CPAIR,
                  sq_engines=SQ_ENGINES, zcopy_act=ZCOPY_ACT):
    """Build the per-core Bacc program. Same program on all cores (SPMD)."""
    import concourse.bacc as bacc
    import concourse.tile as tile
    from concourse import mybir

    f32 = mybir.dt.float32
    f16 = mybir.dt.float16
    i32 = mybir.dt.int32
    AF = mybir.ActivationFunctionType
    ALU = mybir.AluOpType

    nblk = tch // 128
    dsub = D // 128
    nchunks = L_ // tch
    assert tch % 128 == 0 and L_ % tch == 0 and D % 128 == 0

    nc = bacc.Bacc()
    # host-swizzled input: (g, c, p, blk, d) = x[g, c*tch + blk*128 + p, d]
    xin = nc.declare_dram_parameter("xin", [gpc, nchunks, 128, nblk, D], f16,
                                    isOutput=False)
    # host-built diag stationaries, partition-major so the DMA is 128
    # contiguous 16KB rows: (p, g, k, s, m) = diag(w_eff)[g, k, s][p, m]
    wdg = nc.declare_dram_parameter("wdg", [128, gpc, K, dsub, 128], f16,
                                    isOutput=False)
    idn = nc.declare_dram_parameter("idn", [128, 128], f16, isOutput=False)
    # channel-major output: (g, c, p, s, t) = y[g, d=s*128+p, l=c*tch+t]
    yout = nc.declare_dram_parameter("yout", [gpc, nchunks, 128, dsub, tch],
                                     f16, isOutput=True)

    xv = xin[:]
    yv = yout[:]

    with tile.TileContext(nc) as tc:
        with (
            tc.tile_pool(name="pconst", bufs=1) as pconst,
            tc.tile_pool(name="px", bufs=10) as px,
            tc.tile_pool(name="pstat", bufs=4) as pstat,
            tc.tile_pool(name="pz", bufs=5) as pz,
            tc.tile_pool(name="py", bufs=4) as py,
            tc.tile_pool(name="pp1", bufs=2, space="PSUM") as pp1,
            tc.tile_pool(name="pp2", bufs=2, space="PSUM") as pp2,
        ):
            ident = pconst.tile([128, 128], f16)
            nc.scalar.dma_start(out=ident[:], in_=idn[:])
            wsb = pconst.tile([128, gpc, K, dsub, 128], f16)
            nc.scalar.dma_start(out=wsb[:], in_=wdg[:])

            def emit_stats(g, c):
                """Load one chunk + stats + r + drt, emitted a pair ahead of
                the heavy compute so diag(r) never gates PE."""
                xh = px.tile([128, nblk, D], f16, tag="xh")
                nc.sync.dma_start(out=xh[:], in_=xv[g, c])

                ssq = pstat.tile([128, nblk], f32, tag="ssq")
                odd_engines = ("act", "vector", "vector", "act")
                for blk in range(nblk):
                    eng = (sq_engines if c % 2 == 0
                           else odd_engines)[blk % len(sq_engines)]
                    scr = pstat.tile([128, D], f16, tag="scr")
                    if eng == "act":
                        nc.scalar.activation(
                            out=scr[:], in_=xh[:, blk, :],
                            func=AF.Square, scale=float(D) ** -0.5,
                            accum_out=ssq[:, blk:blk + 1])
                    else:
                        nc.vector.scalar_tensor_tensor(
                            out=scr[:], in0=xh[:, blk, :],
                            scalar=1.0 / D, in1=xh[:, blk, :],
                            op0=ALU.mult, op1=ALU.mult,
                            accum_out=ssq[:, blk:blk + 1])
                # r = rsqrt(ms+eps): bit trick + 1 Newton step on
                # DVE (keeps Sqrt out of ACT -> zero table swaps)
                v = pstat.tile([128, nblk], f32, tag="v")
                nc.vector.tensor_scalar(
                    out=v[:], in0=ssq[:], scalar1=EPS, scalar2=None,
                    op0=ALU.add)
                r = pstat.tile([128, nblk], f32, tag="r")
                nc.vector.tensor_scalar(
                    out=r[:].bitcast(i32), in0=v[:].bitcast(i32),
                    scalar1=1, scalar2=None, op0=ALU.arith_shift_right)
                nc.vector.tensor_scalar(
                    out=r[:].bitcast(i32), in0=r[:].bitcast(i32),
                    scalar1=-1, scalar2=0x5F3759DF,
                    op0=ALU.mult, op1=ALU.add)
                yy = pstat.tile([128, nblk], f32, tag="yy")
                nc.vector.tensor_tensor(
                    out=yy[:], in0=r[:], in1=r[:], op=ALU.mult)
                nc.vector.tensor_tensor(
                    out=yy[:], in0=yy[:], in1=v[:], op=ALU.mult)
                nc.vector.tensor_scalar(
                    out=yy[:], in0=yy[:], scalar1=-0.5, scalar2=1.5,
                    op0=ALU.mult, op1=ALU.add)
                rn = pstat.tile([128, nblk], f32, tag="rn")
                nc.vector.tensor_tensor(
                    out=rn[:], in0=r[:], in1=yy[:], op=ALU.mult)

                drt = pstat.tile([128, nblk, 128], f16, tag="drt")
                for blk in range(nblk):
                    nc.vector.tensor_scalar_mul(
                        out=drt[:, blk, :], in0=ident[:],
                        scalar1=rn[:, blk:blk + 1])
                return (xh, drt)

            zt_prev = None
            pair_keys = [(g, c0) for g in range(gpc)
                         for c0 in range(0, nchunks, cpair)]
            pending = [emit_stats(pair_keys[0][0], pair_keys[0][1] + j)
                       for j in range(cpair)]
            for pidx, (g, c0) in enumerate(pair_keys):
                cs = list(range(c0, min(c0 + cpair, nchunks)))
                ncs = len(cs)
                cur = pending
                if pidx + 1 < len(pair_keys):
                    g2, c2 = pair_keys[pidx + 1]
                    pending = [emit_stats(g2, c2 + j) for j in range(cpair)]

                # ---- pass1 per chunk: Z[d, t] = X^T diag(r) ----
                zts = []
                for j, c in enumerate(cs):
                    xh, drt = cur[j]
                    zt = pz.tile([128, dsub, PAD + tch], f16, tag="zt")
                    if c == 0:
                        nc.vector.memset(zt[:, :, 0:PAD], 0.0)
                    else:
                        nc.vector.tensor_copy(
                            out=zt[:, :, 0:PAD],
                            in_=zt_prev[:, :, tch:tch + PAD])
                    for si in range(dsub // 2):
                        zp = pp1.tile([128, 2, tch], f32, tag="zp")
                        for sh in range(2):
                            s = 2 * si + sh
                            for blk in range(nblk):
                                nc.tensor.matmul(
                                    zp[:, sh, blk * 128:(blk + 1) * 128],
                                    lhsT=xh[:, blk, s * 128:(s + 1) * 128],
                                    rhs=drt[:, blk, :],
                                    start=True, stop=True)
                        dst = zt[:, 2 * si:2 * si + 2, PAD:PAD + tch]
                        if si < zcopy_act:
                            nc.scalar.copy(out=dst, in_=zp[:])
                        else:
                            nc.vector.tensor_copy(out=dst, in_=zp[:])
                    zt_prev = zt
                    zts.append(zt)

                # ---- pass2 paired: conv matmuls share ldweights;
                #      silu writes fp16 channel-major, DMA straight out
                for j, c in enumerate(cs):
                    yh = py.tile([128, dsub, tch], f16, tag="yh")
                    for si in range(dsub // 2):
                        yp = pp2.tile([128, 2, tch], f32, tag="yp")
                        for sh in range(2):
                            s = 2 * si + sh
                            for k in range(K):
                                nc.tensor.matmul(
                                    yp[:, sh, :],
                                    lhsT=wsb[:, g, k, s, :],
                                    rhs=zts[j][:, s, k * DIL:k * DIL + tch],
                                    start=(k == 0), stop=(k == K - 1))
                        nc.scalar.activation(
                            out=yh[:, 2 * si:2 * si + 2, :],
                            in_=yp[:], func=AF.Silu)
                        if si == dsub // 4 - 1:
                            nc.gpsimd.dma_start(
                                out=yv[g, c, :, 0:dsub // 2],
                                in_=yh[:, 0:dsub // 2])
                    nc.gpsimd.dma_start(out=yv[g, c, :, dsub // 2:dsub],
                                        in_=yh[:, dsub // 2:dsub])
    nc.compile()
    return nc


def _host_pack(x, norm_weight, conv_weight):
    """Shard inputs across cores; swizzle for contiguous DMA; fold norm
    weight into compact per-channel conv weight columns."""
    dsub = D // 128
    nblk = TCH // 128
    nchunks = L // TCH
    xg = np.ascontiguousarray(x.transpose(0, 2, 1, 3)).reshape(NGROUPS, L, D)
    xg = xg.astype(np.float16)
    # (g, c, p, blk, d) = x[g, c*tch + blk*128 + p, d]
    xsw = np.ascontiguousarray(
        xg.reshape(NGROUPS, nchunks, nblk, 128, D).transpose(0, 1, 3, 2, 4))

    conv_w = conv_weight.reshape(HC, D, K)            # [hc, d, k]
    weff = conv_w * norm_weight[:, :, None]           # [hc, d, k]
    wr = weff.transpose(0, 2, 1).reshape(HC, K, dsub, 128)  # [hc, k, s, p]
    eye = np.eye(128, dtype=np.float32)
    wdiag = (wr[..., None] * eye).astype(np.float16)  # [hc, K, s, p, m]
    idn = np.eye(128, dtype=np.float16)

    in_maps = []
    for i in range(NCORES):
        gs = [i * GPC + j for j in range(GPC)]
        wcore = np.stack([wdiag[g % HC] for g in gs])  # [gpc, K, s, p, m]
        wpm = np.ascontiguousarray(
            wcore.transpose(3, 0, 1, 2, 4))            # [p, gpc, K, s, m]
        in_maps.append({
            "xin": np.ascontiguousarray(xsw[gs[0]:gs[-1] + 1]),
            "wdg": wpm,
            "idn": idn,
        })
    return in_maps


def _host_unpack(results):
    dsub = D // 128
    nchunks = L // TCH
    # yout per core: [gpc, nchunks, 128, dsub, tch] channel-major
    ys = np.concatenate([r["yout"] for r in results], axis=0)
    ys = ys.reshape(B, HC, nchunks, 128, dsub, TCH)
    # [b, hc, c, p, s, t] -> [b, (c t), hc, (s p)]
    y = ys.transpose(0, 2, 5, 1, 4, 3).reshape(B, L, HC, D)
    return np.ascontiguousarray(y.astype(np.float32))


def _get_prog():
    key = (L, GPC, TCH, CPAIR, SQ_ENGINES, ZCOPY_ACT)
    if key not in _prog_cache:
        _prog_cache[key] = build_program()
    return _prog_cache[key]


def kernel(x, norm_weight, conv_weight, _trace=False, _trace_kwargs=None):
    from concourse.bass_utils import run_bass_kernel_spmd

    x = np.asarray(x, dtype=np.float32)
    norm_weight = np.asarray(norm_weight, dtype=np.float32)
    conv_weight = np.asarray(conv_weight, dtype=np.float32)

    nc = _get_prog()
    in_maps = _host_pack(x, norm_weight, conv_weight)
    res = run_bass_kernel_spmd(
        nc, in_maps, list(range(NCORES)),
        trace=_trace, **(_trace_kwargs or {}))
    out = _host_unpack(res.results)
    if _trace:
        return out, res
    return out
